# revision 9
# baseline (speedup 1.0000x reference)
"""MaxPool1d(K=4, stride=2, VALID) over ragged NaN-padded sequences.

Full input:  x  [16, 8, 64, 8192] f32, channel c valid prefix LENS[c], NaN tail.
Full output:    [16, 8, 64, 4095] f32, pooled valid prefix, NaN tail.

Sharding: data-parallel over batch — 16 batches / 8 cores = 2 per core.
Per core, for each channel c the 2 batches x 64 features form exactly 128
partition rows of length LENS[c]; pooling runs along the free dim:
  stage 1: m[i]   = max(x[2i], x[2i+1])      (stride-2 tensor_tensor max)
  stage 2: out[p] = max(m[p], m[p+1])        (unit-stride tensor_tensor max)

DMA strategy (all bulk traffic SWDGE via gpsimd — HWDGE dynamic queues are
serial ~27 GB/s, used only for two dependency-free NaN-tail stores):
  - every transfer is chunked to [128, <=2048] (<=1 MB, 8 KB rows),
  - stage-1 compute runs per loaded chunk so DVE chases the loads,
  - output NaN tails come from a persistent SBUF NaN tile, independent of
    compute,
  - stores write only the valid pooled prefix.
Only the valid input prefix is ever read from HBM.
"""

import numpy as np

# ---- problem constants (hardcoded; kernel.py must be self-contained) ----
B, C, F, S = 16, 8, 64, 8192
K, STRIDE = 4, 2
P_OUT = (S - K) // STRIDE + 1  # 4095
LENS = [8192, 4096, 2048, 1024, 8192, 4096, 2048, 1024]
N_CORES = 8
B_LOC = B // N_CORES  # 2 batches per core
MAX_TAIL = max(P_OUT - ((L - K) // STRIDE + 1) for L in LENS)  # 3584
CK = 2048  # DMA chunk width (columns)

_CACHE = {}


def _chunks(n):
    return [(s, min(CK, n - s)) for s in range(0, n, CK)]


def _build_nc():
    import concourse.bacc as bacc
    import concourse.mybir as mybir
    from concourse.tile import TileContext

    nc = bacc.Bacc("TRN2", debug=False, num_devices=N_CORES)
    x = nc.declare_dram_parameter(
        "x", [B_LOC, C, F, S], mybir.dt.float32, isOutput=False
    )
    out = nc.declare_dram_parameter(
        "out", [B_LOC, C, F, P_OUT], mybir.dt.float32, isOutput=True
    )
    x_ap = x.ap()
    out_ap = out.ap()

    order = [0, 4, 1, 5, 2, 6, 3, 7]  # big channels first

    with TileContext(nc) as tc:
        with tc.tile_pool(name="xin", bufs=1) as in_pool, \
             tc.tile_pool(name="nan", bufs=1) as nan_pool, \
             tc.tile_pool(name="mid", bufs=2) as mid_pool, \
             tc.tile_pool(name="res", bufs=2) as out_pool:

            nan_tile = nan_pool.tile([128, MAX_TAIL], mybir.dt.float32)
            nc.vector.memset(nan_tile[:], float("nan"))

            def emit_nan_tail(c, eng):
                L = LENS[c]
                Pv = (L - K) // STRIDE + 1
                if Pv >= P_OUT:
                    return
                for (s0, w) in _chunks(P_OUT - Pv):
                    eng.dma_start(
                        out=out_ap[:, c, :, Pv + s0:Pv + s0 + w],
                        in_=nan_tile[:, s0:s0 + w],
                    )

            # 1) ALL loads first — per-channel resident tiles, chunked so
            # descriptors spread evenly over the SDMA engines; no slot
            # reuse -> no WAR deps -> the gpsimd stream never stalls here.
            xins = {}
            for c in order:
                L = LENS[c]
                xin = in_pool.tile([128, L], mybir.dt.float32, tag=f"xin{c}")
                for (s0, w) in _chunks(L):
                    nc.gpsimd.dma_start(
                        out=xin[:, s0:s0 + w], in_=x_ap[:, c, :, s0:s0 + w]
                    )
                xins[c] = xin

            # 2) NaN tails, queued behind the loads: two big tails on the
            # otherwise-idle HWDGE rings, the rest on gpsimd.
            emit_nan_tail(1, nc.sync)
            emit_nan_tail(5, nc.scalar)
            for cc in (2, 6, 3, 7):
                emit_nan_tail(cc, nc.gpsimd)

            # 3) pool each channel, store the valid prefix (chunked)
            for c in order:
                L = LENS[c]
                Pv = (L - K) // STRIDE + 1
                xin = xins[c]

                m = mid_pool.tile([128, L // 2], mybir.dt.float32, tag="mid")
                x2 = xin[:].rearrange("p (n two) -> p n two", two=2)
                nc.vector.tensor_max(m[:], x2[:, :, 0], x2[:, :, 1])

                o = out_pool.tile([128, Pv], mybir.dt.float32, tag="res")
                nc.vector.tensor_max(o[:], m[:, 0:Pv], m[:, 1:Pv + 1])
                for (s0, w) in _chunks(Pv):
                    nc.gpsimd.dma_start(
                        out=out_ap[:, c, :, s0:s0 + w], in_=o[:, s0:s0 + w]
                    )
    nc.compile()
    return nc


def _get_nc():
    if "nc" not in _CACHE:
        _CACHE["nc"] = _build_nc()
    return _CACHE["nc"]


def kernel(x: np.ndarray) -> np.ndarray:
    from concourse.bass_utils import run_bass_kernel_spmd

    x = np.asarray(x, dtype=np.float32)
    assert x.shape == (B, C, F, S), x.shape

    nc = _get_nc()
    in_maps = [
        {"x": np.ascontiguousarray(x[i * B_LOC:(i + 1) * B_LOC])}
        for i in range(N_CORES)
    ]
    res = run_bass_kernel_spmd(nc, in_maps, list(range(N_CORES)))
    return np.concatenate([r["out"] for r in res.results], axis=0)


# revision 10
# speedup vs baseline: 1.2825x; 1.2825x over previous
"""MaxPool1d(K=4, stride=2, VALID) over ragged NaN-padded sequences.

Full input:  x  [16, 8, 64, 8192] f32, channel c valid prefix LENS[c], NaN tail.
Full output:    [16, 8, 64, 4095] f32, pooled valid prefix, NaN tail.

Sharding: data-parallel over batch — 16 batches / 8 cores = 2 per core.
Per core, for each channel c the 2 batches x 64 features form exactly 128
partition rows of length LENS[c]; pooling runs along the free dim:
  stage 1: m[i]   = max(x[2i], x[2i+1])      (stride-2 tensor_tensor max)
  stage 2: out[p] = max(m[p], m[p+1])        (unit-stride tensor_tensor max)

DMA strategy (all bulk traffic SWDGE via gpsimd — HWDGE dynamic queues are
serial ~27 GB/s, used only for two dependency-free NaN-tail stores):
  - every transfer is chunked to [128, <=2048] (<=1 MB, 8 KB rows),
  - stage-1 compute runs per loaded chunk so DVE chases the loads,
  - output NaN tails come from a persistent SBUF NaN tile, independent of
    compute,
  - stores write only the valid pooled prefix.
Only the valid input prefix is ever read from HBM.
"""

import numpy as np

# ---- problem constants (hardcoded; kernel.py must be self-contained) ----
B, C, F, S = 16, 8, 64, 8192
K, STRIDE = 4, 2
P_OUT = (S - K) // STRIDE + 1  # 4095
LENS = [8192, 4096, 2048, 1024, 8192, 4096, 2048, 1024]
N_CORES = 8
B_LOC = B // N_CORES  # 2 batches per core
MAX_TAIL = max(P_OUT - ((L - K) // STRIDE + 1) for L in LENS)  # 3584
CK = 2048  # DMA chunk width (columns)

_CACHE = {}


def _chunks(n):
    return [(s, min(CK, n - s)) for s in range(0, n, CK)]


def _build_nc():
    import concourse.bacc as bacc
    import concourse.mybir as mybir
    from concourse.tile import TileContext

    nc = bacc.Bacc("TRN2", debug=False, num_devices=N_CORES)
    x = nc.declare_dram_parameter(
        "x", [B_LOC, C, F, S], mybir.dt.float32, isOutput=False
    )
    out = nc.declare_dram_parameter(
        "out", [B_LOC, C, F, P_OUT], mybir.dt.float32, isOutput=True
    )
    x_ap = x.ap()
    out_ap = out.ap()

    order = [0, 4, 1, 5, 2, 6, 3, 7]  # big channels first

    with TileContext(nc) as tc:
        with tc.tile_pool(name="xin", bufs=1) as in_pool, \
             tc.tile_pool(name="nan", bufs=1) as nan_pool, \
             tc.tile_pool(name="mid", bufs=2) as mid_pool, \
             tc.tile_pool(name="res", bufs=2) as out_pool:

            nan_tile = nan_pool.tile([128, MAX_TAIL], mybir.dt.float32)
            nc.vector.memset(nan_tile[:], float("nan"))

            def emit_nan_tail(c, eng):
                L = LENS[c]
                Pv = (L - K) // STRIDE + 1
                if Pv >= P_OUT:
                    return
                for (s0, w) in _chunks(P_OUT - Pv):
                    eng.dma_start(
                        out=out_ap[:, c, :, Pv + s0:Pv + s0 + w],
                        in_=nan_tile[:, s0:s0 + w],
                    )

            # 1) ALL loads first — per-channel resident tiles, chunked so
            # descriptors spread evenly over the SDMA engines; no slot
            # reuse -> no WAR deps -> the gpsimd stream never stalls here.
            xins = {}
            for c in order:
                L = LENS[c]
                xin = in_pool.tile([128, L], mybir.dt.float32, tag=f"xin{c}")
                for (s0, w) in _chunks(L):
                    nc.gpsimd.dma_start(
                        out=xin[:, s0:s0 + w], in_=x_ap[:, c, :, s0:s0 + w]
                    )
                xins[c] = xin

            # 2) NaN tails, queued behind the loads — all on gpsimd: the
            # serial HWDGE rings (~27 GB/s) hold a shared completion-sem
            # lane for ~74us and stall every later DMA emission.
            for cc in (1, 5, 2, 6, 3, 7):
                emit_nan_tail(cc, nc.gpsimd)

            # 3) pool each channel, store the valid prefix (chunked)
            for c in order:
                L = LENS[c]
                Pv = (L - K) // STRIDE + 1
                xin = xins[c]

                m = mid_pool.tile([128, L // 2], mybir.dt.float32, tag="mid")
                x2 = xin[:].rearrange("p (n two) -> p n two", two=2)
                nc.vector.tensor_max(m[:], x2[:, :, 0], x2[:, :, 1])

                o = out_pool.tile([128, Pv], mybir.dt.float32, tag="res")
                nc.vector.tensor_max(o[:], m[:, 0:Pv], m[:, 1:Pv + 1])
                for (s0, w) in _chunks(Pv):
                    nc.gpsimd.dma_start(
                        out=out_ap[:, c, :, s0:s0 + w], in_=o[:, s0:s0 + w]
                    )
    nc.compile()
    return nc


def _get_nc():
    if "nc" not in _CACHE:
        _CACHE["nc"] = _build_nc()
    return _CACHE["nc"]


def kernel(x: np.ndarray) -> np.ndarray:
    from concourse.bass_utils import run_bass_kernel_spmd

    x = np.asarray(x, dtype=np.float32)
    assert x.shape == (B, C, F, S), x.shape

    nc = _get_nc()
    in_maps = [
        {"x": np.ascontiguousarray(x[i * B_LOC:(i + 1) * B_LOC])}
        for i in range(N_CORES)
    ]
    res = run_bass_kernel_spmd(nc, in_maps, list(range(N_CORES)))
    return np.concatenate([r["out"] for r in res.results], axis=0)


# revision 11
# speedup vs baseline: 2.1193x; 1.6525x over previous
"""MaxPool1d(K=4, stride=2, VALID) over ragged NaN-padded sequences.

Full input:  x  [16, 8, 64, 8192] f32, channel c valid prefix LENS[c], NaN tail.
Full output:    [16, 8, 64, 4095] f32, pooled valid prefix, NaN tail.

Sharding: data-parallel over batch — 16 batches / 8 cores = 2 per core.

Layout trick: adjacent channel PAIRS (0,1), (2,3), (4,5), (6,7) are
contiguous in DRAM for a fixed batch, so a [128, W] SBUF tile whose
partition dim is (2 channels x 64 features) maps to a fully CONTIGUOUS
DRAM block — strided row reads/writes (which run ~3x slower than
contiguous on the SDMA engines) disappear:
  - big pairs (0,1)/(4,5): load the full [128, 8192] block (reading the
    NaN tail of the odd channel costs less than a strided prefix read),
  - small pairs (2,3)/(6,7): load [128, 2048] (prefix of both channels,
    strided, but only 1 MB),
  - every store is one contiguous [128, 4095] block: valid prefix computed
    in place, NaN tails pre-memset into persistent per-pair output tiles.
Pooling per pair-tile, partition-split by channel half:
  stage 1: m[i] = max(x[2i], x[2i+1])   stride-2 tensor_tensor max, 128 rows
  stage 2: out[p] = max(m[p], m[p+1])   unit-stride, per 64-row half
All DMA via gpsimd SWDGE (HWDGE dynamic rings are serial ~27 GB/s).
"""

import numpy as np

# ---- problem constants (hardcoded; kernel.py must be self-contained) ----
B, C, F, S = 16, 8, 64, 8192
K, STRIDE = 4, 2
P_OUT = (S - K) // STRIDE + 1  # 4095
LENS = [8192, 4096, 2048, 1024, 8192, 4096, 2048, 1024]
N_CORES = 8
B_LOC = B // N_CORES  # 2 batches per core

_CACHE = {}


def _pv(L):
    return (L - K) // STRIDE + 1


def _build_nc():
    import concourse.bacc as bacc
    import concourse.mybir as mybir
    from concourse.tile import TileContext

    f32 = mybir.dt.float32
    nc = bacc.Bacc("TRN2", debug=False, num_devices=N_CORES)
    x = nc.declare_dram_parameter("x", [B_LOC, C, F, S], f32, isOutput=False)
    out = nc.declare_dram_parameter("out", [B_LOC, C, F, P_OUT], f32, isOutput=True)
    x_ap = x.ap()
    out_ap = out.ap()

    # (pair base channel, columns to load, load is full-contiguous?)
    PAIRS = {0: (8192, True), 4: (8192, True), 2: (2048, False), 6: (2048, False)}
    # process order: batch-interleaved, big pairs first
    WORK = [(0, 0), (4, 0), (0, 1), (4, 1), (2, 0), (6, 0), (2, 1), (6, 1)]

    with TileContext(nc) as tc:
        with tc.tile_pool(name="big", bufs=2) as big_pool, \
             tc.tile_pool(name="small", bufs=2) as small_pool, \
             tc.tile_pool(name="mid", bufs=2) as mid_pool, \
             tc.tile_pool(name="res", bufs=1) as res_pool:

            # persistent per-pair output tiles with NaN tails pre-set
            otiles = {}
            for cp, (_, _) in PAIRS.items():
                o = res_pool.tile([128, P_OUT], f32, tag=f"o{cp}")
                for half, c in ((0, cp), (1, cp + 1)):
                    tail0 = _pv(LENS[c])
                    if tail0 < P_OUT:
                        nc.vector.memset(
                            o[64 * half:64 * half + 64, tail0:P_OUT], float("nan")
                        )
                otiles[cp] = o

            for cp, b in WORK:
                W, full = PAIRS[cp]
                pool = big_pool if full else small_pool
                xin = pool.tile([128, W], f32, tag="big" if full else "small")
                # [2 channels, 64 features, W] -> 128 partitions; contiguous
                # in DRAM when full (W == S)
                nc.gpsimd.dma_start(out=xin[:], in_=x_ap[b, cp:cp + 2, :, 0:W])

                m = mid_pool.tile([128, W // 2], f32, tag="midb" if full else "mids")
                x2 = xin[:].rearrange("p (n two) -> p n two", two=2)
                nc.vector.tensor_max(m[:], x2[:, :, 0], x2[:, :, 1])

                o = otiles[cp]
                for half, c in ((0, cp), (1, cp + 1)):
                    Pv = _pv(LENS[c])
                    r0 = 64 * half
                    nc.vector.tensor_max(
                        o[r0:r0 + 64, 0:Pv],
                        m[r0:r0 + 64, 0:Pv],
                        m[r0:r0 + 64, 1:Pv + 1],
                    )
                # one contiguous [128, P_OUT] store covering both channels
                nc.gpsimd.dma_start(out=out_ap[b, cp:cp + 2, :, :], in_=o[:])
    nc.compile()
    return nc


def _get_nc():
    if "nc" not in _CACHE:
        _CACHE["nc"] = _build_nc()
    return _CACHE["nc"]


def kernel(x: np.ndarray) -> np.ndarray:
    from concourse.bass_utils import run_bass_kernel_spmd

    x = np.asarray(x, dtype=np.float32)
    assert x.shape == (B, C, F, S), x.shape

    nc = _get_nc()
    in_maps = [
        {"x": np.ascontiguousarray(x[i * B_LOC:(i + 1) * B_LOC])}
        for i in range(N_CORES)
    ]
    res = run_bass_kernel_spmd(nc, in_maps, list(range(N_CORES)))
    return np.concatenate([r["out"] for r in res.results], axis=0)
